# revision 16
# baseline (speedup 1.0000x reference)
"""Trainium2 Bass kernel for CPELayer_ResAG (concept-routed LoRA edit layer).

Computation (per token t with concept c = concept_idx[t]):
    down = edit_direction[t] @ lora_down[c]          # [768]@[768,4] -> [4]
    up   = down @ lora_up[c]                         # [4]@[4,1280]  -> [1280]
    out  = x[t] @ W.T + b_lin + 0.25 * up

Strategy: data-parallel over batch across 8 cores (616 tokens/core, padded
to 640 = 5 x 128 token tiles).  The routed LoRA runs densely over all 50
concepts: A.T[(c,r), t] = lora_down_flat.T @ ed.T, masked on-device with a
one-hot built from is_equal against the broadcast token->concept ids (the
MoE routing), then contracted with lora_up_flat on the tensor engine into
the same PSUM as the org matmul.  The bias rides along as one extra
contraction row (a constant row in the masked operand, b_lin row in luB).

Perf notes (vs the fp32r v1 at 51.3us):
 - org operands and the output are bf16 (same 1 cycle/row on the PE as
   fp32r, half the HBM bytes); the whole LoRA branch (ed, lora_down x64,
   lora_up x8) is fp8e4 -- it contributes ~0.7% of the output scale, so
   fp8 rounding there is invisible.  Scales are exact powers of two,
   undone by the one-hot mask value (2^-9) and the x0.125 bias row.
 - A.T and the up-matmul run fp8 DoubleRow: 256 contraction rows/pass.
 - DMA traffic is packed host-side into a few [128, L] blobs with fat
   (2.5-11.5KB) contiguous partition lines: SDMA descriptor overhead
   (~150-240ns each) makes thin-line transfers run at a fraction of the
   ~360 GB/s HBM rate (v2's 200B-line lora_down load alone cost ~6us,
   a [128, 2] fp32 load ~2us).
 - loads split across BOTH HWDGE rings (sync + scalar), critical-path
   first: the A.T operands land first, then lu/x/W for wave A.
 - token->concept ids and the concept row-values broadcast/transpose via
   K=1 matmuls from one single-line DMA instead of per-partition loads.
 - dummy warmup matmuls during the load phase pull the HAM clock gate to
   2.4 GHz before the real matmuls start.

All host-side work is layout only (pad / transpose / reshape / dtype
casts); every FLOP of the reference runs on device.
"""

import sys
import types

import numpy as np

import concourse.mybir as mybir
import concourse.tile as tile
from concourse import bacc
from concourse.bass_utils import run_bass_kernel_spmd

# If BASS_TRACE is set in the environment, run_bass_kernel_spmd imports
# antenv.axon_hooks, which some containers lack; stub it (None hook ->
# tracing is skipped gracefully, execution unaffected).
try:
    import antenv.axon_hooks  # noqa: F401
except ImportError:
    _m = types.ModuleType("antenv.axon_hooks")
    _m.get_axon_ntff_profile_hook = lambda: None
    _m.set_axon_ntff_profile_hook = lambda h: None
    sys.modules["antenv.axon_hooks"] = _m

# Problem shapes (hardcoded per spec nn_CPELayer_ResAG_19335942766951)
N_CORES = 8
B, T, DIN, DOUT = 64, 77, 768, 1280
N_CONCEPTS, RANK = 50, 4
SCALE = 0.25                # alpha/rank = 1/4, exact power of two
BPC = B // N_CORES          # batches per core = 8
TOK = BPC * T               # real tokens per core = 616
TOKP = 640                  # padded tokens = 5 full 128-token tiles
NT = TOKP // 128            # 5 token tiles
NJ = N_CONCEPTS * RANK      # 200 flattened (concept, rank) rows
NJP = 208                   # j padded so the DoubleRow plane stride is %16
P = 128
KD = DIN // P               # 6 k-tiles of the d_in contraction
KA = 3                      # org k-tiles in wave A (k0..2), wave B k3..5
NH = TOKP // 2              # 320-token halves for the A.T psum tiles
LD_SCALE = 64.0             # fp8 pre-scale on lora_down
LU_SCALE = 8.0              # fp8 pre-scale on 0.25*lora_up (and bias)
MASK_VAL = 1.0 / (LD_SCALE * LU_SCALE)   # 2^-9, exact in bf16
BIAS_VAL = 1.0 / LU_SCALE                # 0.125, exact in fp8
N_CHUNKS = [(0, 512), (512, 512), (1024, 256)]
IDX_PAD = -2.0              # pad-token id; cvals pad is -1 so never equal
N_WARM = 4
DR = mybir.MatmulPerfMode.DoubleRow

# fp8 pack layout (per-partition offsets, elements == bytes)
LD_LEN = KD * NJP           # 1248
ED_LEN = TOKP               # 640 per k-tile
L8A = LD_LEN + 2 * ED_LEN   # pack a: ld + ed k0..1   (2528 B lines)
L8B = 4 * ED_LEN            # pack b: ed k2..5        (2560 B lines)
# bf16 pack for wave B (elements)
PBX = (KD - KA) * TOKP      # 1920 xb elements
PBW = (KD - KA) * DOUT      # 3840 Wb elements
IDXCV = TOKP + 2 * P        # idx row + cv row, single line

_cache = {}


def _build_bass():
    nc = bacc.Bacc("TRN2", target_bir_lowering=False, debug=False,
                   num_devices=N_CORES)
    f32 = mybir.dt.float32
    bf = mybir.dt.bfloat16
    f8 = mybir.dt.float8e4

    # All DRAM inputs are pre-swizzled host-side to the SBUF image:
    # partition dim first, contiguous fat lines.
    l8a_d = nc.dram_tensor("l8a", [P, L8A], f8, kind="ExternalInput").ap()
    l8b_d = nc.dram_tensor("l8b", [P, L8B], f8, kind="ExternalInput").ap()
    lu_d = nc.dram_tensor("luB", [P, 2 * DOUT], f8, kind="ExternalInput").ap()
    xa_d = nc.dram_tensor("xa", [P, KA * TOKP], bf, kind="ExternalInput").ap()
    Wa_d = nc.dram_tensor("Wa", [P, KA * DOUT], bf, kind="ExternalInput").ap()
    pb_d = nc.dram_tensor("pb", [P, PBX + PBW], bf, kind="ExternalInput").ap()
    idxcv_d = nc.dram_tensor("idxcv", [1, IDXCV], bf,
                             kind="ExternalInput").ap()
    out_d = nc.dram_tensor("out", [TOKP, DOUT], bf, kind="ExternalOutput").ap()

    with tile.TileContext(nc) as tc:
        with (
            tc.tile_pool(name="consts", bufs=1) as consts,
            tc.tile_pool(name="outsb", bufs=5) as outsb,
        ):
            # ---- input DMAs first in program order (= issue priority).
            # Two HWDGE rings, each FIFO.  sync: idxcv, l8a, l8b, Wa, pb;
            # scalar: lu, xa, then the output stores.
            idxcv = consts.tile([1, IDXCV], bf, tag="idxcv")
            nc.sync.dma_start(idxcv[:], idxcv_d[:, :])
            l8 = consts.tile([P, L8A + L8B], f8, tag="l8")
            nc.sync.dma_start(l8[:, 0:L8A], l8a_d[:, :])
            lu_fl = consts.tile([P, 2 * DOUT], f8, tag="lu")
            nc.scalar.dma_start(lu_fl[:], lu_d[:, :])
            nc.sync.dma_start(l8[:, L8A:L8A + L8B], l8b_d[:, :])
            xa_fl = consts.tile([P, KA * TOKP], bf, tag="xa")
            nc.scalar.dma_start(xa_fl[:], xa_d[:, :])
            Wa_fl = consts.tile([P, KA * DOUT], bf, tag="Wa")
            nc.sync.dma_start(Wa_fl[:], Wa_d[:, :])
            pb = consts.tile([P, PBX + PBW], bf, tag="pb")
            nc.sync.dma_start(pb[:], pb_d[:, :])

            # typed views into the packs
            ld3 = l8[:, 0:LD_LEN].rearrange("p (k j) -> p k j", k=KD)
            ed01 = l8[:, LD_LEN:L8A].rearrange("p (k t) -> p k t", k=2)
            ed25 = l8[:, L8A:L8A + L8B].rearrange("p (k t) -> p k t", k=4)
            ed_pair = [ed01[:, 0:2, :], ed25[:, 0:2, :], ed25[:, 2:4, :]]
            lu_all = lu_fl.rearrange("p (j o) -> p j o", j=2)
            xa = xa_fl.rearrange("p (k t) -> p k t", k=KA)
            Wa = Wa_fl.rearrange("p (k o) -> p k o", k=KA)
            xb = pb[:, 0:PBX].rearrange("p (k t) -> p k t", k=KD - KA)
            Wb = pb[:, PBX:PBX + PBW].rearrange("p (k o) -> p k o", k=KD - KA)

            warm = consts.tile([P, 512], bf, tag="warm")
            nc.vector.memset(warm[:], 0.0)
            ones = consts.tile([1, P], bf, tag="ones")
            nc.vector.memset(ones[:], 1.0)
            ones1 = consts.tile([1, 1], bf, tag="ones1")
            nc.vector.memset(ones1[:], 1.0)
            cvals = consts.tile([P, 2], f32, tag="cvals")

            masks = [consts.tile([P, TOKP], bf, tag=f"mask{j}",
                                 name=f"mask{j}") for j in range(2)]
            # MT holds both 128-row j-planes side by side for DoubleRow.
            MT = consts.tile([P, 2, TOKP], f8, tag="MT")
            # Plane-1 rows 72..127 pair with luB rows 200..255: zero them,
            # then the bias row at 96 carries 1/LU_SCALE (b_lin*LU_SCALE
            # sits at luB[224]); the mask-mul overwrites rows 0..71.
            nc.vector.memset(MT[64:P, 1, :], 0.0)
            nc.vector.memset(MT[96:97, 1, :], BIAS_VAL)

            with tc.tile_pool(name="warm_ps", bufs=1, space="PSUM") as wpool:
                wps = wpool.tile([P, 512], f32, tag="wps")
                for _ in range(N_WARM):
                    nc.tensor.matmul(wps[:], warm[:, 0:P], warm[:],
                                     start=True, stop=True)

            # Transpose the concept row-values (one K=1 matmul per 128-row
            # half) and broadcast the token ids (K=1, ones stationary);
            # then build the one-hot masks.  MASK_VAL undoes the fp8
            # scales on lora_down/lora_up.
            with tc.tile_pool(name="idx_ps", bufs=4, space="PSUM") as ipool:
                for jc in range(2):
                    cvp = ipool.tile([P, 1], f32, tag="cvp")
                    nc.tensor.matmul(
                        cvp[:], idxcv[:, TOKP + jc * P:TOKP + (jc + 1) * P],
                        ones1[:], start=True, stop=True)
                    nc.any.tensor_copy(out=cvals[:, jc:jc + 1], in_=cvp[:])
                for nh in range(2):
                    nsl = slice(nh * NH, (nh + 1) * NH)
                    ips = ipool.tile([P, NH], f32, tag="ips")
                    nc.tensor.matmul(ips[:], ones[:], idxcv[:, nsl],
                                     start=True, stop=True)
                    for jc in range(2):
                        nc.vector.tensor_scalar(
                            masks[jc][:, nsl], ips[:],
                            cvals[:, jc:jc + 1], MASK_VAL,
                            mybir.AluOpType.is_equal, mybir.AluOpType.mult)

            # A.T = (64*lora_down_flat).T @ ed.T for all concepts, three
            # DoubleRow passes of 256 contraction rows, masked into MT.
            # kk outer with 4 open psum groups, so the kk=0 pass runs as
            # soon as the first pack lands while ed k2..5 is in flight.
            with tc.tile_pool(name="at_ps", bufs=4, space="PSUM") as at_pool:
                ats = {}
                for jc in range(2):
                    jp = P if jc == 0 else NJ - P  # 128, 72
                    for nh in range(2):
                        ats[jc, nh] = at_pool.tile([P, NH], f32, tag="at",
                                                   name=f"at{jc}{nh}")
                for kk in range(KD // 2):
                    for jc in range(2):
                        jp = P if jc == 0 else NJ - P
                        jsl = slice(jc * P, jc * P + jp)
                        for nh in range(2):
                            nsl = slice(nh * NH, (nh + 1) * NH)
                            nc.tensor.matmul(
                                ats[jc, nh][:jp, :],
                                ld3[:, 2 * kk:2 * kk + 2, jsl],
                                ed_pair[kk][:, :, nsl],
                                start=(kk == 0), stop=(kk == KD // 2 - 1),
                                perf_mode=DR)
                for jc in range(2):
                    jp = P if jc == 0 else NJ - P
                    for nh in range(2):
                        nsl = slice(nh * NH, (nh + 1) * NH)
                        nc.vector.tensor_tensor(
                            MT[:jp, jc, nsl], ats[jc, nh][:jp, :],
                            masks[jc][:jp, nsl], mybir.AluOpType.mult)

            # Main accumulation, two waves per (t, n) so PSUM banks recycle
            # while the late W/x k-tiles are still in flight:
            #   wave A: up (one DoubleRow matmul) + org k0..2 -> copy osb
            #   wave B: org k3..5 -> DVE-add into osb -> DMA out
            # The stationary loop is outer, n-chunks inner, so consecutive
            # matmuls share the stationary operand.
            osbs = []
            with tc.tile_pool(name="out_ps", bufs=8, space="PSUM") as out_pool:
                for ti in range(NT):
                    tsl = slice(ti * P, (ti + 1) * P)
                    osb = outsb.tile([P, DOUT], bf, tag="osb")
                    osbs.append(osb)
                    ps3 = [out_pool.tile([P, 512], f32, tag="ops",
                                         name=f"psA{ci}")
                           for ci in range(len(N_CHUNKS))]
                    nmm = 1 + KA
                    for i in range(nmm):
                        for ci, (n0, nw) in enumerate(N_CHUNKS):
                            if i == 0:
                                nc.tensor.matmul(
                                    ps3[ci][:, :nw], MT[:, :, tsl],
                                    lu_all[:, :, n0:n0 + nw],
                                    start=True, stop=False, perf_mode=DR)
                            else:
                                k = i - 1
                                nc.tensor.matmul(
                                    ps3[ci][:, :nw], xa[:, k, tsl],
                                    Wa[:, k, n0:n0 + nw],
                                    start=False, stop=(i == nmm - 1))
                    for ci, (n0, nw) in enumerate(N_CHUNKS):
                        nc.any.tensor_copy(out=osb[:, n0:n0 + nw],
                                           in_=ps3[ci][:, :nw])
                for ti in range(NT):
                    tsl = slice(ti * P, (ti + 1) * P)
                    osb = osbs[ti]
                    ps3 = [out_pool.tile([P, 512], f32, tag="ops",
                                         name=f"psB{ci}")
                           for ci in range(len(N_CHUNKS))]
                    for i, k in enumerate(range(KA, KD)):
                        for ci, (n0, nw) in enumerate(N_CHUNKS):
                            nc.tensor.matmul(
                                ps3[ci][:, :nw], xb[:, k - KA, tsl],
                                Wb[:, k - KA, n0:n0 + nw],
                                start=(i == 0), stop=(i == KD - KA - 1))
                    for ci, (n0, nw) in enumerate(N_CHUNKS):
                        nc.any.tensor_tensor(
                            osb[:, n0:n0 + nw], ps3[ci][:, :nw],
                            osb[:, n0:n0 + nw], mybir.AluOpType.add)
                        if ti == NT - 1:
                            nc.scalar.dma_start(out_d[tsl, n0:n0 + nw],
                                                osb[:, n0:n0 + nw])
                    if ti != NT - 1:
                        nc.scalar.dma_start(out_d[tsl, :], osb[:, :])

    nc.compile()
    return nc


def get_bass():
    if "v4" not in _cache:
        _cache["v4"] = _build_bass()
    return _cache["v4"]


def _swz(a, k0, k1):
    """[DIN, L] -> [P, (k1-k0)*L] SBUF-image lines for k-tiles k0..k1."""
    return a.reshape(KD, P, -1)[k0:k1].transpose(1, 0, 2).reshape(P, -1)


def make_in_maps(x, edit_direction, concept_idx, lora_down, lora_up, W, b_lin):
    """Host-side sharding + layout prep (no reference FLOPs)."""
    bf = mybir.dt.np(mybir.dt.bfloat16)
    f8 = mybir.dt.np(mybir.dt.float8e4)
    x = np.asarray(x, dtype=np.float32)
    ed = np.asarray(edit_direction, dtype=np.float32)
    idx = np.asarray(concept_idx)
    ld = np.asarray(lora_down, dtype=np.float32)
    lup = np.asarray(lora_up, dtype=np.float32)
    W = np.asarray(W, dtype=np.float32)
    b = np.asarray(b_lin, dtype=np.float32)

    WT = W.T                                                     # [768, 1280]
    Wa = np.ascontiguousarray(_swz(WT, 0, KA).astype(bf))
    Wb = _swz(WT, KA, KD)
    ldT = np.zeros((DIN, NJP), dtype=np.float32)
    ldT[:, :NJ] = ld.transpose(1, 0, 2).reshape(DIN, NJ) * LD_SCALE
    ld_sw = _swz(ldT, 0, KD)                                     # [P, 1248]
    luB = np.zeros((2 * P, DOUT), dtype=np.float32)
    luB[:NJ] = lup.reshape(NJ, DOUT) * (SCALE * LU_SCALE)
    luB[128 + 96] = b * LU_SCALE                                 # bias row
    luB = np.ascontiguousarray(
        luB.reshape(2, P, DOUT).transpose(1, 0, 2).reshape(P, -1).astype(f8))
    cvrow = np.full(2 * P, -1.0, dtype=np.float32)
    cvrow[:NJ] = np.arange(NJ, dtype=np.float32) // RANK

    in_maps = []
    for c in range(N_CORES):
        sl = slice(c * BPC, (c + 1) * BPC)
        xs = np.zeros((TOKP, DIN), dtype=np.float32)
        xs[:TOK] = x[sl].reshape(TOK, DIN)
        xT = xs.T                                                # [768, 640]
        eds = np.zeros((TOKP, DIN), dtype=np.float32)
        eds[:TOK] = ed[sl].reshape(TOK, DIN)
        edT = eds.T
        ed_sw = _swz(edT, 0, KD)                                 # [P, 3840]
        idxcv = np.full(IDXCV, IDX_PAD, dtype=np.float32)
        idxcv[:TOK] = idx[sl].reshape(TOK).astype(np.float32)
        idxcv[TOKP:] = cvrow
        l8a = np.concatenate([ld_sw, ed_sw[:, :2 * ED_LEN]], axis=1)
        pbm = np.concatenate([_swz(xT, KA, KD), Wb], axis=1)
        in_maps.append({
            "l8a": np.ascontiguousarray(l8a.astype(f8)),
            "l8b": np.ascontiguousarray(ed_sw[:, 2 * ED_LEN:].astype(f8)),
            "luB": luB,
            "xa": np.ascontiguousarray(_swz(xT, 0, KA).astype(bf)),
            "Wa": Wa,
            "pb": np.ascontiguousarray(pbm.astype(bf)),
            "idxcv": np.ascontiguousarray(
                idxcv.reshape(1, IDXCV).astype(bf)),
        })
    return in_maps


def kernel(x, edit_direction, concept_idx, lora_down, lora_up, W, b_lin,
           _trace=False, _mm_dtype=None, _lora_dtype=None):
    nc = get_bass()
    in_maps = make_in_maps(x, edit_direction, concept_idx, lora_down,
                           lora_up, W, b_lin)
    res = run_bass_kernel_spmd(nc, in_maps, core_ids=list(range(N_CORES)),
                               trace=_trace)
    out = np.concatenate(
        [r["out"][:TOK].astype(np.float32) for r in res.results], axis=0)
    out = out.reshape(B, T, DOUT)
    if _trace:
        kernel.last_results = res
    return out
